# revision 18
# baseline (speedup 1.0000x reference)
"""Trainium2 Bass kernel for the delayed-dense spiking network.

Network (reference semantics):
    s1 = spike(delayed_dense(psp(x),  w1, d1))   # [B, 800, T]
    s3 = spike(delayed_dense(psp(s1), w3, d3))   # [B, 10, T]

psp is a linear causal filter (u[t] = a*u[t-1] + s[t]) and delayed_dense is a
shift-grouped GEMM; psp commutes exactly with the time shifts and (up to fp
rounding ~1e-6 rel) with the channel mixing, so we evaluate each layer as
    spike(psp(sum_s W_s @ shift_s(x)))
which lets both GEMMs run on *binary* activations (exact in bf16).  Delays lie
in [0,4) so only shifts 0..4 are live (5 shift matrices).

Sharding: data-parallel over batch, 8 batch elements per NeuronCore.

Layer 1 packs all 5 shifts along the contraction axis entirely in fp8e4m3
under DoubleRow (32 subtiles of 128 -> 16 paired-subtile matmuls at 2
contraction rows per cycle).  The binary input is exact in fp8; the fp8
weight quantization was verified (direct numpy fp64 simulation) to leave the
final output bit-identical.  Layer 2 stays bf16 (its weight error perturbs
h3 directly with no flip cancellation).

Every matmul is lowered to LDWEIGHTS + MATMUL, and a DoubleRow LDWEIGHTS
streams 256 weight columns (~213ns) while an N=350 matmul streams in only
~146ns - with one batch per matmul the PE is weight-load bound.  So the
batch dim is packed into the matmul free axis: 4 batches side by side
(4*352 = 1408 columns), chunked at PSUM-bank-aligned 512/512/384 so each
matmul streams >=384 columns and the weight load of the next matmul hides
behind the current stream.  Two groups of 4 batches keep PSUM inside its
8 banks (2 x 3-bank accumulators double-buffered + 2 banks for layer 2).

(Measured dead ends on this stack: column-group concurrency for the thin
last m-tile serializes and regresses ~30us; closing each chain with a
plain-fp8 matmul on the odd last live k-subtile - skipping the all-zero
32nd subtile - regresses ~28us, apparently from in-chain perf-mode
switching.  Both reverted: all chains are 16 uniform DoubleRow pairs.)

Layer 2 stacks the 5 shift matrices along the output axis (M = 5*10 = 50)
and also runs fp8 DoubleRow (verified: max u3 moves 3.595 -> 3.599, still
6.4 below threshold, zero output spikes either way): 4 uniform DR pairs
over an 8-subtile zero-padded contraction, s1 stored as binary fp8 at a
368-column slot stride and w3t padded to 64 columns (DR step % 16).  The
shifted partials are merged either with 5 small SBUF->SBUF DMAs (partition
regroup) plus 4 vector adds, or - for the last batch, where the chain is the
kernel tail - with 5 tiny float32r selector matmuls that keep the merge on
the PE.

Host-side prep:
  - w1f: all-shift masked transposed weights: [4096, 800] fp8e4m3
  - w3t: shift-stacked transposed weights: [1024, 64] fp8e4m3
         (col = s*10 + o, zero-padded)
  - xg:  shift-replicated binary input, 4 batches packed along the free
         axis: [2, 4096, 1408] fp8e4m3
  - sel: shift-unstack selector, sel[s*10+o, s, o] = 1: [50, 5, 10] f32
"""

import numpy as np
import ml_dtypes

NIN, NHID, NOUT = 784, 800, 10
B, T = 64, 350
NSHIFT = 5            # delays in [0,4) touch integer shifts 0..4
TAU = 10.0
THETA = 10.0
DMAX = 4.0
N_CORES = 8
BPC = B // N_CORES    # batches per core
GB = 4                # batches per group (packed along matmul free axis)
NG = BPC // GB        # groups per core
KF_TILES = 32         # all 5 shifts in fp8: ceil(5*784/128)=31, padded even
KF_PAD = KF_TILES * 128   # 4096
TF = 352              # per-batch slot width: DoubleRow needs offsets % 16 == 0
GW = GB * TF          # 1408: group width (4 batches side by side)
CHUNKS = ((0, 512), (512, 1024), (1024, GW))   # PSUM-bank-aligned N chunks
PH_W = 1536           # psum accumulator width: 3 full banks of 512
NIN_PAD = 896         # 7*128
NHID_PAD = 896        # 7*128
M1_TILES = 7          # ceil(800/128)
K2_TILES = 7          # ceil(800/128)
TW = T + 4            # layer-2 merge width with 4 leading zero columns
TW2 = 368             # s1 slot width: DoubleRow needs the subtile step % 16
K2P_TILES = 8         # layer-2 contraction padded to 8 subtiles (4 DR pairs)
NHID_PAD2 = K2P_TILES * 128   # 1024
M2 = NSHIFT * NOUT    # 50
M2P = 64              # layer-2 lhsT free width: DoubleRow step % 16

DECAY = float(np.float32(np.exp(np.float64(-1.0 / TAU))))

_BF16 = ml_dtypes.bfloat16


def _masked_shift_weights(w, d):
    """Return list of NSHIFT float32 [O, I] shift matrices (linear interp)."""
    d = np.clip(d.astype(np.float32), 0.0, np.float32(DMAX))
    fl = np.floor(d)
    frac = d - fl
    out = []
    for s in range(NSHIFT):
        ws = w * ((fl == s).astype(np.float32) * (1.0 - frac)
                  + (fl == (s - 1)).astype(np.float32) * frac)
        out.append(ws.astype(np.float32))
    return out


def _prep_host(spike_input, w1, d1, w3, d3):
    import ml_dtypes as _md
    w1s = _masked_shift_weights(w1, d1)           # 5 x [800, 784]
    w1f = np.zeros((KF_PAD, NHID), dtype=_md.float8_e4m3)
    for s in range(NSHIFT):
        w1f[s * NIN:(s + 1) * NIN, :] = w1s[s].T.astype(_md.float8_e4m3)

    w3s = _masked_shift_weights(w3, d3)           # 5 x [10, 800]
    w3t = np.zeros((NHID_PAD2, M2P), dtype=_md.float8_e4m3)
    for s in range(NSHIFT):
        w3t[:NHID, s * NOUT:(s + 1) * NOUT] = w3s[s].T.astype(_md.float8_e4m3)

    xf8 = spike_input.astype(_md.float8_e4m3)     # binary -> exact in fp8
    # xg[g][s*NIN + i, b*TF + t] = x[GB*g + b, i, t - s]
    xg = np.zeros((B // GB, KF_PAD, GW), dtype=_md.float8_e4m3)
    for g in range(B // GB):
        for b in range(GB):
            xb = xf8[g * GB + b]
            for s in range(NSHIFT):
                xg[g, s * NIN:(s + 1) * NIN, b * TF + s:b * TF + T] = \
                    xb[:, :T - s]

    sel = np.zeros((M2, NSHIFT, NOUT), dtype=np.float32)
    for s in range(NSHIFT):
        for o in range(NOUT):
            sel[s * NOUT + o, s, o] = 1.0
    return xg, w1f, w3t, sel


def _build_nc(n_batch=BPC, rep=1, b0_chunked=True):
    import contextlib
    import concourse.bacc as bacc
    import concourse.mybir as mybir
    import concourse.tile as tile

    f32 = mybir.dt.float32
    bf16 = mybir.dt.bfloat16

    nc = bacc.Bacc(None, target_bir_lowering=False, debug=False)
    f8 = mybir.dt.float8e4
    xg_d = nc.dram_tensor("xg", [NG, KF_PAD, GW], f8, kind="ExternalInput")
    w1f_d = nc.dram_tensor("w1f", [KF_PAD, NHID], f8, kind="ExternalInput")
    w3t_d = nc.dram_tensor("w3t", [NHID_PAD2, M2P], f8, kind="ExternalInput")
    sel_d = nc.dram_tensor("sel", [M2, NSHIFT, NOUT], f32, kind="ExternalInput")
    out_d = nc.dram_tensor("out", [n_batch, NOUT, T], f32, kind="ExternalOutput")

    with tile.TileContext(nc) as tc:
        with (
            tc.tile_pool(name="const", bufs=1) as constp,
            tc.tile_pool(name="xpool", bufs=2) as xpool,
            tc.tile_pool(name="s1pool", bufs=BPC) as s1pool,
            tc.tile_pool(name="upool", bufs=3) as upool,
            tc.tile_pool(name="qpool", bufs=2) as qpool,
            tc.tile_pool(name="opool", bufs=4) as opool,
            tc.tile_pool(name="psum1", bufs=2, space="PSUM") as psum1,
            tc.tile_pool(name="psum2", bufs=2, space="PSUM") as psum2,
        ):
            w1f = constp.tile([128, KF_TILES, NHID], mybir.dt.float8e4)
            w1f_src = w1f_d.rearrange("(k p) m -> p k m", p=128)
            w3t = constp.tile([128, K2P_TILES, M2P], mybir.dt.float8e4)
            dec = constp.tile([128, T], f32)
            sel_f = constp.tile([M2, NSHIFT, NOUT], f32)
            sel_r = constp.tile([M2, NSHIFT, NOUT], mybir.dt.float32r)

            def _emit_consts():
                nc.scalar.dma_start(w3t[:], w3t_d.rearrange("(k p) c -> p k c", p=128))
                nc.vector.memset(dec[:], DECAY)
                nc.scalar.dma_start(sel_f[:], sel_d[:])
                nc.vector.tensor_copy(sel_r[:], sel_f[:])

            loop_ctx = (
                tc.For_i(0, rep, 1, hint_engines=(mybir.EngineType.PE,))
                if rep > 1 else contextlib.nullcontext()
            )
            with loop_ctx:
                _emit_body(nc, tc, n_batch, xpool, s1pool, upool,
                           qpool, opool, psum1, psum2, xg_d, out_d,
                           w1f, w1f_src, w3t, dec, sel_r,
                           mybir, load_w1t=True, emit_consts=_emit_consts)

    nc.compile()
    return nc


def _emit_l2(nc, mybir, b, s1b, psum2, qpool, opool, dec, w3t, sel_r, out_d,
             tail=False, p3=None):
    """Layer 2 for one batch: M-stacked shift GEMM + partial merge + psp."""
    f32 = mybir.dt.float32
    f32r = mybir.dt.float32r
    mult, add = mybir.AluOpType.mult, mybir.AluOpType.add
    if p3 is None:
        p3 = psum2.tile([M2P, TW2], f32, tag="p3", name=f"p3_{b}")
        for j in range(0, K2P_TILES, 2):
            nc.tensor.matmul(
                p3[:], w3t[:, j:j + 2, :], s1b[:, j:j + 2, :],
                start=(j == 0), stop=(j == K2P_TILES - 2),
                perf_mode=mybir.MatmulPerfMode.DoubleRow,
            )
    if tail:
        # Keep the merge on the PE: float32r selector matmuls read the
        # shifted partial slices and accumulate h3 directly in PSUM.
        q50r = qpool.tile([M2, TW], f32r, tag="q50r")
        nc.vector.tensor_copy(q50r[:], p3[:M2, :TW])
        h3p = psum2.tile([M2P, TW2], f32, tag="p3", name=f"h3p_{b}")
        for s in range(NSHIFT):
            nc.tensor.matmul(
                h3p[:NOUT, :T], sel_r[:, s, :], q50r[:, 4 - s:TW - s],
                start=(s == 0), stop=(s == NSHIFT - 1),
            )
        u3 = opool.tile([NOUT, T], f32, tag="u3", name=f"u3_{b}")
        nc.vector.tensor_tensor_scan(
            u3[:], dec[:NOUT, :], h3p[:NOUT, :T], 0.0, mult, add)
    else:
        q50 = qpool.tile([M2, TW], f32, tag="q50")
        nc.vector.tensor_copy(q50[:], p3[:M2, :TW])
        q = qpool.tile([NOUT, NSHIFT, TW], f32, tag="q")
        dma_engines = [nc.scalar, nc.sync, nc.scalar, nc.sync, nc.scalar]
        for s in range(NSHIFT):
            dma_engines[s].dma_start(q[:, s, :], q50[s * NOUT:(s + 1) * NOUT, :])
        acc = opool.tile([NOUT, T], f32, tag="acc")
        nc.vector.tensor_add(acc[:], q[:, 0, 4:TW], q[:, 1, 3:TW - 1])
        nc.vector.tensor_add(acc[:], acc[:], q[:, 2, 2:TW - 2])
        nc.vector.tensor_add(acc[:], acc[:], q[:, 3, 1:TW - 3])
        nc.vector.tensor_add(acc[:], acc[:], q[:, 4, 0:TW - 4])
        u3 = opool.tile([NOUT, T], f32, tag="u3", name=f"u3_{b}")
        nc.vector.tensor_tensor_scan(u3[:], dec[:NOUT, :], acc[:], 0.0, mult, add)
    o3 = opool.tile([NOUT, T], f32, tag="o3", name=f"o3_{b}")
    nc.vector.tensor_scalar(
        out=o3[:], in0=u3[:], scalar1=THETA, scalar2=None,
        op0=mybir.AluOpType.is_ge,
    )
    nc.sync.dma_start(out_d[b], o3[:])


def _emit_body(nc, tc, n_batch, xpool, s1pool, upool, qpool, opool,
               psum1, psum2, xg_d, out_d, w1f,
               w1f_src, w3t, dec, sel_r,
               mybir, load_w1t=True, emit_consts=None):
    f32 = mybir.dt.float32
    bf16 = mybir.dt.bfloat16
    mult, add = mybir.AluOpType.mult, mybir.AluOpType.add
    is_ge = mybir.AluOpType.is_ge

    s1_tiles = [None] * n_batch

    for g in range(NG):
        xg = xpool.tile([128, KF_TILES, GW], mybir.dt.float8e4,
                        tag="xg", name=f"xg_{g}")
        src = xg_d[g].rearrange("(k p) c -> p k c", p=128)
        if g == 0:
            # k-chunked, interleaved with the weight load, so the PE's first
            # accumulation chain starts as soon as the first k-pair lands.
            for k in range(KF_TILES):
                if load_w1t:
                    nc.sync.dma_start(w1f[:, k, :], w1f_src[:, k, :])
                nc.scalar.dma_start(xg[:, k, :], src[:, k, :])
                if k == 0 and emit_consts is not None:
                    emit_consts()
        else:
            nc.scalar.dma_start(xg[:, :16, :], src[:, :16, :])
            nc.sync.dma_start(xg[:, 16:, :], src[:, 16:, :])

        s1g = []
        for b in range(GB):
            t = s1pool.tile([128, K2P_TILES, TW2], mybir.dt.float8e4,
                            tag="s1b", name=f"s1b_{g}_{b}")
            nc.vector.memset(t[:], 0.0)
            s1g.append(t)

        for m in range(M1_TILES):
            mw = min(128, NHID - m * 128)
            ph = psum1.tile([128, PH_W], f32, tag="ph", name=f"ph_{g}_{m}")
            for j in range(0, KF_TILES, 2):
                for (c0, c1) in CHUNKS:
                    nc.tensor.matmul(
                        ph[:mw, c0:c1], w1f[:, j:j + 2, m * 128:m * 128 + mw],
                        xg[:, j:j + 2, c0:c1],
                        start=(j == 0), stop=(j == KF_TILES - 2),
                        perf_mode=mybir.MatmulPerfMode.DoubleRow,
                    )
            for b in range(GB):
                u = upool.tile([128, T], bf16, tag="u", name=f"u_{g}_{m}_{b}")
                nc.vector.tensor_tensor_scan(
                    u[:mw, :], dec[:mw, :], ph[:mw, b * TF:b * TF + T],
                    0.0, mult, add)
                nc.vector.tensor_scalar(
                    out=s1g[b][:mw, m, 4:TW], in0=u[:mw, :],
                    scalar1=THETA, scalar2=None, op0=is_ge,
                )
            # Interleave the previous group's layer 2 under this group's
            # layer-1 matmuls.
            if g > 0 and 1 <= m <= GB:
                bprev = GB * (g - 1) + (m - 1)
                _emit_l2(nc, mybir, bprev, s1_tiles[bprev], psum2, qpool,
                         opool, dec, w3t, sel_r, out_d)
        for b in range(GB):
            s1_tiles[GB * g + b] = s1g[b]

    # Tail: layer 2 for the last group.
    for b in range(GB * (NG - 1), n_batch):
        _emit_l2(nc, mybir, b, s1_tiles[b], psum2, qpool,
                 opool, dec, w3t, sel_r, out_d, tail=(b == n_batch - 1))


def make_in_maps(spike_input, w1, d1, w3, d3):
    xg, w1f, w3t, sel = _prep_host(spike_input, w1, d1, w3, d3)
    in_maps = []
    for c in range(N_CORES):
        in_maps.append({
            "xg": np.ascontiguousarray(xg[c * NG:(c + 1) * NG]),
            "w1f": w1f,
            "w3t": w3t,
            "sel": sel,
        })
    return in_maps


def kernel(spike_input, w1, d1, w3, d3):
    from concourse import bass_utils

    spike_input = np.asarray(spike_input, dtype=np.float32)
    w1 = np.asarray(w1, dtype=np.float32)
    d1 = np.asarray(d1, dtype=np.float32)
    w3 = np.asarray(w3, dtype=np.float32)
    d3 = np.asarray(d3, dtype=np.float32)

    nc = _build_nc()
    in_maps = make_in_maps(spike_input, w1, d1, w3, d3)
    res = bass_utils.run_bass_kernel_spmd(nc, in_maps, core_ids=list(range(N_CORES)))
    out = np.concatenate([res.results[c]["out"] for c in range(N_CORES)], axis=0)
    return out.astype(np.float32)


# revision 19
# speedup vs baseline: 1.4276x; 1.4276x over previous
"""Trainium2 Bass kernel for the delayed-dense spiking network.

Network (reference semantics):
    s1 = spike(delayed_dense(psp(x),  w1, d1))   # [B, 800, T]
    s3 = spike(delayed_dense(psp(s1), w3, d3))   # [B, 10, T]

psp is a linear causal filter (u[t] = a*u[t-1] + s[t]) and delayed_dense is a
shift-grouped GEMM; psp commutes exactly with the time shifts and (up to fp
rounding ~1e-6 rel) with the channel mixing, so we evaluate each layer as
    spike(psp(sum_s W_s @ shift_s(x)))
which lets both GEMMs run on *binary* activations (exact in bf16).  Delays lie
in [0,4) so only shifts 0..4 are live (5 shift matrices).

Sharding: data-parallel over batch, 8 batch elements per NeuronCore.

Layer 1 packs all 5 shifts along the contraction axis entirely in fp8e4m3
under DoubleRow (32 subtiles of 128 -> 16 paired-subtile matmuls at 2
contraction rows per cycle).  The binary input is exact in fp8; the fp8
weight quantization was verified (direct numpy fp64 simulation) to leave the
final output bit-identical.  Layer 2 stays bf16 (its weight error perturbs
h3 directly with no flip cancellation).

Every matmul is lowered to LDWEIGHTS + MATMUL, and a DoubleRow LDWEIGHTS
streams 256 weight columns (~213ns) while an N=350 matmul streams in only
~146ns - with one batch per matmul the PE is weight-load bound.  So the
batch dim is packed into the matmul free axis: 4 batches side by side
(4*352 = 1408 columns), chunked at PSUM-bank-aligned 512/512/384 so each
matmul streams >=384 columns and the weight load of the next matmul hides
behind the current stream.  Two groups of 4 batches keep PSUM inside its
8 banks (2 x 3-bank accumulators double-buffered + 2 banks for layer 2).

(Measured dead ends on this stack: column-group concurrency for the thin
last m-tile serializes and regresses ~30us; closing each chain with a
plain-fp8 matmul on the odd last live k-subtile - skipping the all-zero
32nd subtile - regresses ~28us, apparently from in-chain perf-mode
switching.  Both reverted: all chains are 16 uniform DoubleRow pairs.)

Layer 2 stacks the 5 shift matrices along the output axis (M = 5*10 = 50)
and also runs fp8 DoubleRow (verified: max u3 moves 3.595 -> 3.599, still
6.4 below threshold, zero output spikes either way): 4 uniform DR pairs
over an 8-subtile zero-padded contraction, s1 stored as binary fp8 at a
368-column slot stride and w3t padded to 64 columns (DR step % 16).  The
shifted partials are merged either with 5 small SBUF->SBUF DMAs (partition
regroup) plus 4 vector adds, or - for the last batch, where the chain is the
kernel tail - with 5 tiny float32r selector matmuls that keep the merge on
the PE.

Host-side prep:
  - w1f: all-shift masked transposed weights: [4096, 800] fp8e4m3
  - w3t: shift-stacked transposed weights: [1024, 64] fp8e4m3
         (col = s*10 + o, zero-padded)
  - xg:  shift-replicated binary input, 4 batches packed along the free
         axis: [2, 4096, 1408] fp8e4m3
  - sel: shift-unstack selector, sel[s*10+o, s, o] = 1: [50, 5, 10] f32
"""

import numpy as np
import ml_dtypes

NIN, NHID, NOUT = 784, 800, 10
B, T = 64, 350
NSHIFT = 5            # delays in [0,4) touch integer shifts 0..4
TAU = 10.0
THETA = 10.0
DMAX = 4.0
N_CORES = 8
BPC = B // N_CORES    # batches per core
GB = 4                # batches per group (packed along matmul free axis)
NG = BPC // GB        # groups per core
KF_TILES = 32         # all 5 shifts in fp8: ceil(5*784/128)=31, padded even
KF_PAD = KF_TILES * 128   # 4096
TF = 352              # per-batch slot width: DoubleRow needs offsets % 16 == 0
GW = GB * TF          # 1408: group width (4 batches side by side)
CHUNKS = ((0, 512), (512, 1024), (1024, GW))   # PSUM-bank-aligned N chunks
PH_W = 1536           # psum accumulator width: 3 full banks of 512
NIN_PAD = 896         # 7*128
NHID_PAD = 896        # 7*128
M1_TILES = 7          # ceil(800/128)
K2_TILES = 7          # ceil(800/128)
TW = T + 4            # layer-2 merge width with 4 leading zero columns
TW2 = 368             # s1 slot width: DoubleRow needs the subtile step % 16
K2P_TILES = 8         # layer-2 contraction padded to 8 subtiles (4 DR pairs)
NHID_PAD2 = K2P_TILES * 128   # 1024
M2 = NSHIFT * NOUT    # 50
M2P = 64              # layer-2 lhsT free width: DoubleRow step % 16

DECAY = float(np.float32(np.exp(np.float64(-1.0 / TAU))))

_BF16 = ml_dtypes.bfloat16


def _masked_shift_weights(w, d):
    """Return list of NSHIFT float32 [O, I] shift matrices (linear interp)."""
    d = np.clip(d.astype(np.float32), 0.0, np.float32(DMAX))
    fl = np.floor(d)
    frac = d - fl
    out = []
    for s in range(NSHIFT):
        ws = w * ((fl == s).astype(np.float32) * (1.0 - frac)
                  + (fl == (s - 1)).astype(np.float32) * frac)
        out.append(ws.astype(np.float32))
    return out


def _prep_host(spike_input, w1, d1, w3, d3):
    import ml_dtypes as _md
    w1s = _masked_shift_weights(w1, d1)           # 5 x [800, 784]
    w1f = np.zeros((KF_PAD, NHID), dtype=_md.float8_e4m3)
    for s in range(NSHIFT):
        w1f[s * NIN:(s + 1) * NIN, :] = w1s[s].T.astype(_md.float8_e4m3)

    w3s = _masked_shift_weights(w3, d3)           # 5 x [10, 800]
    w3t = np.zeros((NHID_PAD2, M2P), dtype=_md.float8_e4m3)
    for s in range(NSHIFT):
        w3t[:NHID, s * NOUT:(s + 1) * NOUT] = w3s[s].T.astype(_md.float8_e4m3)

    xf8 = spike_input.astype(_md.float8_e4m3)     # binary -> exact in fp8
    # xg[g][s*NIN + i, b*TF + t] = x[GB*g + b, i, t - s]
    xg = np.zeros((B // GB, KF_PAD, GW), dtype=_md.float8_e4m3)
    for g in range(B // GB):
        for b in range(GB):
            xb = xf8[g * GB + b]
            for s in range(NSHIFT):
                xg[g, s * NIN:(s + 1) * NIN, b * TF + s:b * TF + T] = \
                    xb[:, :T - s]

    sel = np.zeros((M2, NSHIFT, NOUT), dtype=np.float32)
    for s in range(NSHIFT):
        for o in range(NOUT):
            sel[s * NOUT + o, s, o] = 1.0
    return xg, w1f, w3t, sel


def _build_nc(n_batch=BPC, rep=1, b0_chunked=True):
    import contextlib
    import concourse.bacc as bacc
    import concourse.mybir as mybir
    import concourse.tile as tile

    f32 = mybir.dt.float32
    bf16 = mybir.dt.bfloat16

    nc = bacc.Bacc(None, target_bir_lowering=False, debug=False)
    f8 = mybir.dt.float8e4
    xg_d = nc.dram_tensor("xg", [NG, KF_PAD, GW], f8, kind="ExternalInput")
    w1f_d = nc.dram_tensor("w1f", [KF_PAD, NHID], f8, kind="ExternalInput")
    w3t_d = nc.dram_tensor("w3t", [NHID_PAD2, M2P], f8, kind="ExternalInput")
    sel_d = nc.dram_tensor("sel", [M2, NSHIFT, NOUT], f32, kind="ExternalInput")
    out_d = nc.dram_tensor("out", [n_batch, NOUT, T], f32, kind="ExternalOutput")

    with tile.TileContext(nc) as tc:
        with (
            tc.tile_pool(name="const", bufs=1) as constp,
            tc.tile_pool(name="xpool", bufs=2) as xpool,
            tc.tile_pool(name="s1pool", bufs=BPC) as s1pool,
            tc.tile_pool(name="upool", bufs=3) as upool,
            tc.tile_pool(name="qpool", bufs=2) as qpool,
            tc.tile_pool(name="opool", bufs=4) as opool,
            tc.tile_pool(name="psum1", bufs=2, space="PSUM") as psum1,
            tc.tile_pool(name="psum2", bufs=2, space="PSUM") as psum2,
        ):
            w1f = constp.tile([128, KF_TILES, NHID], mybir.dt.float8e4)
            w1f_src = w1f_d.rearrange("(k p) m -> p k m", p=128)
            w3t = constp.tile([128, K2P_TILES, M2P], mybir.dt.float8e4)
            dec = constp.tile([128, T], f32)
            sel_f = constp.tile([M2, NSHIFT, NOUT], f32)
            sel_r = constp.tile([M2, NSHIFT, NOUT], mybir.dt.float32r)

            def _emit_consts():
                nc.scalar.dma_start(w3t[:], w3t_d.rearrange("(k p) c -> p k c", p=128))
                nc.vector.memset(dec[:], DECAY)
                nc.scalar.dma_start(sel_f[:], sel_d[:])
                nc.vector.tensor_copy(sel_r[:], sel_f[:])

            loop_ctx = (
                tc.For_i(0, rep, 1, hint_engines=(mybir.EngineType.PE,))
                if rep > 1 else contextlib.nullcontext()
            )
            with loop_ctx:
                _emit_body(nc, tc, n_batch, xpool, s1pool, upool,
                           qpool, opool, psum1, psum2, xg_d, out_d,
                           w1f, w1f_src, w3t, dec, sel_r,
                           mybir, load_w1t=True, emit_consts=_emit_consts)

    nc.compile()
    return nc


def _emit_l2(nc, mybir, b, s1b, psum2, qpool, opool, dec, w3t, sel_r, out_d,
             tail=False, p3=None):
    """Layer 2 for one batch: M-stacked shift GEMM + partial merge + psp."""
    f32 = mybir.dt.float32
    f32r = mybir.dt.float32r
    mult, add = mybir.AluOpType.mult, mybir.AluOpType.add
    if p3 is None:
        p3 = psum2.tile([M2P, TW2], f32, tag="p3", name=f"p3_{b}")
        for j in range(0, K2P_TILES, 2):
            nc.tensor.matmul(
                p3[:], w3t[:, j:j + 2, :], s1b[:, j:j + 2, :],
                start=(j == 0), stop=(j == K2P_TILES - 2),
                perf_mode=mybir.MatmulPerfMode.DoubleRow,
            )
    if tail:
        # Keep the merge on the PE: float32r selector matmuls read the
        # shifted partial slices and accumulate h3 directly in PSUM.
        q50r = qpool.tile([M2, TW], f32r, tag="q50r")
        nc.vector.tensor_copy(q50r[:], p3[:M2, :TW])
        h3p = psum2.tile([M2P, TW2], f32, tag="p3", name=f"h3p_{b}")
        for s in range(NSHIFT):
            nc.tensor.matmul(
                h3p[:NOUT, :T], sel_r[:, s, :], q50r[:, 4 - s:TW - s],
                start=(s == 0), stop=(s == NSHIFT - 1),
            )
        u3 = opool.tile([NOUT, T], f32, tag="u3", name=f"u3_{b}")
        nc.vector.tensor_tensor_scan(
            u3[:], dec[:NOUT, :], h3p[:NOUT, :T], 0.0, mult, add)
    else:
        q50 = qpool.tile([M2, TW], f32, tag="q50")
        nc.vector.tensor_copy(q50[:], p3[:M2, :TW])
        q = qpool.tile([NOUT, NSHIFT, TW], f32, tag="q")
        dma_engines = [nc.scalar, nc.sync, nc.scalar, nc.sync, nc.scalar]
        for s in range(NSHIFT):
            dma_engines[s].dma_start(q[:, s, :], q50[s * NOUT:(s + 1) * NOUT, :])
        acc = opool.tile([NOUT, T], f32, tag="acc")
        nc.vector.tensor_add(acc[:], q[:, 0, 4:TW], q[:, 1, 3:TW - 1])
        nc.vector.tensor_add(acc[:], acc[:], q[:, 2, 2:TW - 2])
        nc.vector.tensor_add(acc[:], acc[:], q[:, 3, 1:TW - 3])
        nc.vector.tensor_add(acc[:], acc[:], q[:, 4, 0:TW - 4])
        u3 = opool.tile([NOUT, T], f32, tag="u3", name=f"u3_{b}")
        nc.vector.tensor_tensor_scan(u3[:], dec[:NOUT, :], acc[:], 0.0, mult, add)
    o3 = opool.tile([NOUT, T], f32, tag="o3", name=f"o3_{b}")
    nc.vector.tensor_scalar(
        out=o3[:], in0=u3[:], scalar1=THETA, scalar2=None,
        op0=mybir.AluOpType.is_ge,
    )
    nc.sync.dma_start(out_d[b], o3[:])


def _emit_body(nc, tc, n_batch, xpool, s1pool, upool, qpool, opool,
               psum1, psum2, xg_d, out_d, w1f,
               w1f_src, w3t, dec, sel_r,
               mybir, load_w1t=True, emit_consts=None):
    f32 = mybir.dt.float32
    bf16 = mybir.dt.bfloat16
    mult, add = mybir.AluOpType.mult, mybir.AluOpType.add
    is_ge = mybir.AluOpType.is_ge

    s1_tiles = [None] * n_batch

    for g in range(NG):
        xg = xpool.tile([128, KF_TILES, GW], mybir.dt.float8e4,
                        tag="xg", name=f"xg_{g}")
        src = xg_d[g].rearrange("(k p) c -> p k c", p=128)
        if g == 0:
            # k-chunked, interleaved with the weight load, so the PE's first
            # accumulation chain starts as soon as the first k-pair lands.
            for k in range(KF_TILES):
                if load_w1t and k % 2 == 0 and k // 2 < M1_TILES:
                    m = k // 2
                    mw = min(128, NHID - m * 128)
                    nc.sync.dma_start(w1f[:, :, m * 128:m * 128 + mw],
                                      w1f_src[:, :, m * 128:m * 128 + mw])
                nc.scalar.dma_start(xg[:, k, :], src[:, k, :])
                if k == 0 and emit_consts is not None:
                    emit_consts()
        else:
            nc.scalar.dma_start(xg[:, :16, :], src[:, :16, :])
            nc.sync.dma_start(xg[:, 16:, :], src[:, 16:, :])

        s1g = []
        for b in range(GB):
            t = s1pool.tile([128, K2P_TILES, TW2], mybir.dt.float8e4,
                            tag="s1b", name=f"s1b_{g}_{b}")
            nc.vector.memset(t[:], 0.0)
            s1g.append(t)

        for m in range(M1_TILES):
            mw = min(128, NHID - m * 128)
            ph = psum1.tile([128, PH_W], f32, tag="ph", name=f"ph_{g}_{m}")
            for j in range(0, KF_TILES, 2):
                for (c0, c1) in CHUNKS:
                    nc.tensor.matmul(
                        ph[:mw, c0:c1], w1f[:, j:j + 2, m * 128:m * 128 + mw],
                        xg[:, j:j + 2, c0:c1],
                        start=(j == 0), stop=(j == KF_TILES - 2),
                        perf_mode=mybir.MatmulPerfMode.DoubleRow,
                    )
            for b in range(GB):
                u = upool.tile([128, T], bf16, tag="u", name=f"u_{g}_{m}_{b}")
                nc.vector.tensor_tensor_scan(
                    u[:mw, :], dec[:mw, :], ph[:mw, b * TF:b * TF + T],
                    0.0, mult, add)
                nc.vector.tensor_scalar(
                    out=s1g[b][:mw, m, 4:TW], in0=u[:mw, :],
                    scalar1=THETA, scalar2=None, op0=is_ge,
                )
            # Interleave the previous group's layer 2 under this group's
            # layer-1 matmuls.
            if g > 0 and 1 <= m <= GB:
                bprev = GB * (g - 1) + (m - 1)
                _emit_l2(nc, mybir, bprev, s1_tiles[bprev], psum2, qpool,
                         opool, dec, w3t, sel_r, out_d)
        for b in range(GB):
            s1_tiles[GB * g + b] = s1g[b]

    # Tail: layer 2 for the last group.
    for b in range(GB * (NG - 1), n_batch):
        _emit_l2(nc, mybir, b, s1_tiles[b], psum2, qpool,
                 opool, dec, w3t, sel_r, out_d, tail=False)


def make_in_maps(spike_input, w1, d1, w3, d3):
    xg, w1f, w3t, sel = _prep_host(spike_input, w1, d1, w3, d3)
    in_maps = []
    for c in range(N_CORES):
        in_maps.append({
            "xg": np.ascontiguousarray(xg[c * NG:(c + 1) * NG]),
            "w1f": w1f,
            "w3t": w3t,
            "sel": sel,
        })
    return in_maps


def kernel(spike_input, w1, d1, w3, d3):
    from concourse import bass_utils

    spike_input = np.asarray(spike_input, dtype=np.float32)
    w1 = np.asarray(w1, dtype=np.float32)
    d1 = np.asarray(d1, dtype=np.float32)
    w3 = np.asarray(w3, dtype=np.float32)
    d3 = np.asarray(d3, dtype=np.float32)

    nc = _build_nc()
    in_maps = make_in_maps(spike_input, w1, d1, w3, d3)
    res = bass_utils.run_bass_kernel_spmd(nc, in_maps, core_ids=list(range(N_CORES)))
    out = np.concatenate([res.results[c]["out"] for c in range(N_CORES)], axis=0)
    return out.astype(np.float32)
